# revision 18
# baseline (speedup 1.0000x reference)
"""CTC loss kernel for Trainium2, 8-way data parallel over the batch.

Per core (32 examples): the CTC forward DP runs s-major — for each extended
state s (193), the time recursion is one `tensor_tensor_scan` per t-segment
on DVE, batched over examples on partitions.  T is processed as 256+128+128
with a power-of-2 renorm between segments (exponent extracted with int ALU
ops, so the renorm never queues behind the activation engine).

Alpha rows live in 16 rolling slots of width 515:
  col 0        = seg-0 boundary (0),   cols 1..256   seg-0 out (t 0..255)
  col 257      = renormed alpha[255],  cols 258..385 seg-1 out (t 256..383)
  col 386      = renormed alpha[383],  cols 387..514 seg-2 out (t 384..511)
A row's shifted window is contiguous inside its slot, so no per-row boundary
copies are needed; boundary values cross the renorm via one bulk capture and
one bulk scatter per 8 rows ([32,8] strided copies).

Emissions are blank-normalized (E = exp(x[label] - x[blank] + ln_kappa)); the
log-softmax denominator cancels up to a bulk sum of per-(b,t) logsumexp done
via ACT exp + PE ones-matmuls + one ACT ln.  Gathered label logits come from
Δ-one-hot matmuls (bf16) on PE; the [l,e,t]→[e,l,t] layout flip is a batched
DRAM-roundtrip DMA per 24-row chunk (direct SBUF→SBUF permuted DMAs corrupt
data on this toolchain — do not "simplify" back to one DMA).
"""
import numpy as np
from contextlib import ExitStack

B, T, C, L = 256, 512, 128, 96
S = 2 * L + 1
NCORES = 8
BC = B // NCORES          # 32 examples per core
TS = T // 2               # 256: t-range per emission-production half
# sweep segments: 256 + 128 + 128 with a pow2 renorm between each
# (a single mid renorm is not enough: worst-case end-state decay over a
# 256-step tail exceeds the whole fp32 range below the anchor)
SEGS = [(0, 0, 256), (1, 256, 128), (2, 384, 128)]   # (idx, t0, len)
W = 3 + 256 + 128 + 128   # slot: [b0 |256| b1 |128| b2 |128] = 515 cols
NSLOT = 16
LCH = 24                  # emission-production chunk: label rows per chunk
NCH = L // LCH            # 4 chunks per segment
LN_KAPPA = -1.25
LN2 = float(np.log(2.0))
ROFF = 44                 # renorm target exponent: boundary max -> [2^44, 2^45)
                          # (end-state rows decay ~e^-110 in the second half;
                          # without the offset they underflow fp32)
VPRE = 96                 # readout pre-scale 2^96 so ln() sees a normal value

_cache = {}


def _build():
    import concourse.bass as bass
    import concourse.bacc as bacc
    import concourse.tile as tile
    import concourse.mybir as mybir

    f32 = mybir.dt.float32
    bf16 = mybir.dt.bfloat16
    i32 = mybir.dt.int32
    add = mybir.AluOpType.add
    mult = mybir.AluOpType.mult
    AF = mybir.ActivationFunctionType

    nc = bacc.Bacc("TRN2", target_bir_lowering=False, debug=False,
                   num_devices=NCORES)

    xT_d = nc.dram_tensor("xT", [C, BC, T], bf16, kind="ExternalInput")
    oh_d = nc.dram_tensor("oh", [C, BC, L], bf16, kind="ExternalInput")
    skm_d = nc.dram_tensor("skm", [BC, L], f32, kind="ExternalInput")
    sel_d = nc.dram_tensor("sel", [C, BC], f32, kind="ExternalInput")
    ebh_d = nc.dram_tensor("ebh", [BC, LCH * TS], bf16, kind="ExternalInput")
    out_d = nc.dram_tensor("dev_out", [BC, 1], f32, kind="ExternalOutput")
    dbg_d = nc.dram_tensor("dbg", [BC, 8], f32, kind="ExternalOutput")
    es_d = [nc.dram_tensor(f"es{q}", [L, BC, TS], bf16, kind="Internal")
            for q in range(2)]

    with tile.TileContext(nc, num_cores=NCORES) as tc, ExitStack() as ctx:
        persist = ctx.enter_context(tc.tile_pool(name="persist", bufs=1))
        xtpool = ctx.enter_context(tc.tile_pool(name="xt", bufs=2))
        espool = ctx.enter_context(tc.tile_pool(name="es", bufs=2))
        ebpool = ctx.enter_context(tc.tile_pool(name="eb", bufs=4))
        expool = ctx.enter_context(tc.tile_pool(name="ex", bufs=2))
        upool = ctx.enter_context(tc.tile_pool(name="u", bufs=2))
        psumG = ctx.enter_context(
            tc.tile_pool(name="psumG", bufs=2, space=bass.MemorySpace.PSUM))
        psumS = ctx.enter_context(
            tc.tile_pool(name="psumS", bufs=1, space=bass.MemorySpace.PSUM))
        psumQ = ctx.enter_context(
            tc.tile_pool(name="psumQ", bufs=2, space=bass.MemorySpace.PSUM))

        # ---- persistent tiles ----
        SLOTS = persist.tile([BC, NSLOT * W], f32)
        SV = SLOTS[:].rearrange("p (n w) -> p n w", w=W)
        Z = persist.tile([BC, TS], f32)
        KT = persist.tile([BC, TS], f32)
        ONES = persist.tile([BC, 1], f32)
        SKM = persist.tile([BC, L], f32)
        EC = persist.tile([BC, S], f32)         # alpha[255] per row
        OHALL = persist.tile([C, BC * L], bf16)
        ONESC = persist.tile([C, 1], f32)
        ONESCB = persist.tile([C, 1], bf16)
        KBIAS = persist.tile([LCH, 1], f32)
        SEL = persist.tile([C, BC], f32)
        LNALL = persist.tile([C, C], f32)
        SLQS = persist.tile([C, 1], f32)
        SUMLSE = persist.tile([BC, 1], f32)
        LOGSF = persist.tile([BC, 1], f32)
        SEALL = psumS.tile([C, C], f32)         # lse: col = e*4 + chunk

        nc.vector.memset(Z[:], 0.0)
        nc.vector.memset(KT[:], float(np.exp(np.float32(LN_KAPPA))))
        nc.vector.memset(ONES[:], 1.0)
        nc.vector.memset(ONESC[:], 1.0)
        nc.vector.memset(ONESCB[:], 1.0)
        nc.vector.memset(KBIAS[:], float(LN_KAPPA))
        nc.vector.memset(SV[:, :, 0], 0.0)      # seg-0 boundary cols
        nc.vector.memset(SEALL[:], 0.0)
        nc.sync.dma_start(SKM[:], skm_d[:])
        nc.sync.dma_start(SEL[:], sel_d[:])
        nc.sync.dma_start(OHALL[:].rearrange("c (e l) -> c e l", l=L), oh_d[:])

        XT = []
        for q in range(2):
            xt = xtpool.tile([C, BC * TS], bf16, tag="xt")
            nc.sync.dma_start(xt[:].rearrange("c (e t) -> c e t", t=TS),
                              xT_d[:, :, q * TS:(q + 1) * TS])
            XT.append(xt)

        # ---- emission production: E[e, l, t] chunks via PE gather + ACT exp,
        # reshuffled [l,e,t]->[e,l,t] through DRAM ----
        EB = {}
        # first chunk comes precomputed from the host so the sweep can start
        # without waiting out the on-device production latency chain
        EBH = persist.tile([BC, LCH * TS], bf16)
        nc.sync.dma_start(EBH[:], ebh_d[:])
        EB[(0, 0)] = EBH
        for q in range(2):
            for ch in range(NCH):
                if (q, ch) == (0, 0):
                    continue
                l0 = ch * LCH
                ES = espool.tile([LCH, BC * TS], bf16, tag="es")
                for e0 in range(0, BC, 2):
                    G = psumG.tile([LCH, 2 * TS], f32, tag="G")
                    for j in range(2):
                        e = e0 + j
                        nc.tensor.matmul(
                            G[:, j * TS:(j + 1) * TS],
                            OHALL[:, e * L + l0:e * L + l0 + LCH],
                            XT[q][:, e * TS:(e + 1) * TS],
                            start=True, stop=True)
                    nc.scalar.activation(ES[:, e0 * TS:(e0 + 2) * TS], G[:],
                                         AF.Exp, bias=KBIAS[:])
                nc.sync.dma_start(
                    es_d[q][l0:l0 + LCH],
                    ES[:].rearrange("l (e t) -> l e t", t=TS))
                eb = ebpool.tile([BC, LCH * TS], bf16, tag="eb")
                nc.sync.dma_start(
                    eb[:].rearrange("e (l t) -> e l t", t=TS),
                    es_d[q][l0:l0 + LCH].rearrange("l e t -> e l t"))
                EB[(q, ch)] = eb

        # ---- lse: SEALL[t, e*4 + chunk] = sum_c exp(x[c, t]) ----
        for q in range(2):
            EX = expool.tile([C, BC * TS], bf16, tag="ex")
            for e0 in range(0, BC, 8):
                nc.scalar.activation(EX[:, e0 * TS:(e0 + 8) * TS],
                                     XT[q][:, e0 * TS:(e0 + 8) * TS], AF.Exp)
            for e in range(BC):
                for half in range(2):
                    col = e * 4 + 2 * q + half
                    nc.tensor.matmul(
                        SEALL[:, col:col + 1],
                        EX[:, e * TS + half * C:e * TS + (half + 1) * C],
                        ONESCB[:], start=True, stop=True,
                        skip_group_check=True)

        # ---- the s-sweeps (all DVE) ----
        # slot columns: [b0 | seg0 out | b1 | seg1 out | b2 | seg2 out]
        bases = [0, 257, 386]                   # boundary col per segment

        def sweep(si):
            _, t0, ln = SEGS[si]
            base = bases[si]
            for s in range(S):
                n = s % NSLOT
                if si > 0 and s % 8 == 0:       # scatter renormed boundaries
                    hi = min(s + 8, S)
                    nc.vector.tensor_copy(SV[:, n:n + (hi - s), base],
                                          EC[:, s:hi])
                if s % 2 == 1 and ((s - 1) // 2) % LCH == 0:
                    # absorb the E-chunk DMA dependency into a TT op (the
                    # scan ISA has a single sync-wait slot)
                    ch = ((s - 1) // 2) // LCH
                    nc.vector.tensor_tensor(SKM[:, 0:1], SKM[:, 0:1],
                                            EB[(t0 // TS, ch)][:, 0:1],
                                            mybir.AluOpType.bypass)
                w1 = (SV[:, (s - 1) % NSLOT, base:base + ln] if s >= 1
                      else Z[:, 0:ln])
                w2 = (SV[:, (s - 2) % NSLOT, base:base + ln] if s >= 2
                      else Z[:, 0:ln])
                init = (ONES[:, 0:1] if (si == 0 and s < 2)
                        else SV[:, n, base:base + 1])
                dst = SV[:, n, base + 1:base + 1 + ln]
                if s % 2 == 0:
                    nc.vector.tensor_tensor_scan(dst, w1, KT[:, 0:ln], init,
                                                 add, mult)
                else:
                    l = (s - 1) // 2
                    ch, lo = l // LCH, l % LCH
                    toff = lo * TS + (t0 % TS)
                    U = upool.tile([BC, TS], f32, tag="u")
                    nc.vector.scalar_tensor_tensor(U[:, 0:ln], w2,
                                                   SKM[:, l:l + 1], w1,
                                                   mult, add)
                    nc.vector.tensor_tensor_scan(
                        dst, U[:, 0:ln],
                        EB[(t0 // TS, ch)][:, toff:toff + ln],
                        init, add, mult)
                if si < 2 and (s % 8 == 7 or s == S - 1):
                    lo = s - s % 8              # capture boundary col per row
                    nc.vector.tensor_copy(EC[:, lo:s + 1],
                                          SV[:, lo % NSLOT:lo % NSLOT
                                             + (s + 1 - lo), base + ln])

        # power-of-2 renorm (int ALU only; ACT stays out of the path)
        M = persist.tile([BC, 1], f32)
        EXPB = persist.tile([BC, 1], i32)
        T2 = persist.tile([BC, 1], i32)
        T3 = persist.tile([BC, 1], i32)
        R32 = persist.tile([BC, 1], f32)
        EF = persist.tile([BC, 1], f32)
        LTMP = persist.tile([BC, 1], f32)

        def renorm(first):
            nc.vector.tensor_reduce(M[:], EC[:, 0:S], mybir.AxisListType.X,
                                    mybir.AluOpType.max)
            nc.vector.tensor_scalar(EXPB[:], M[:].bitcast(i32), 23, None,
                                    mybir.AluOpType.logical_shift_right)
            nc.vector.tensor_scalar(T2[:], EXPB[:], -1, 254 + ROFF, mult, add)
            nc.vector.tensor_scalar(T3[:], T2[:], 23, None,
                                    mybir.AluOpType.logical_shift_left)
            nc.vector.tensor_copy(R32[:], T3[:].bitcast(f32))  # 2^(127-e+ROFF)
            nc.vector.tensor_copy(EF[:], EXPB[:])              # int -> float
            dstl = LOGSF[:] if first else LTMP[:]
            nc.vector.tensor_scalar(dstl, EF[:], LN2, -(127.0 + ROFF) * LN2,
                                    mult, add)
            if not first:
                nc.vector.tensor_tensor(LOGSF[:], LOGSF[:], LTMP[:], add)
            nc.vector.tensor_scalar(EC[:, 0:S], EC[:, 0:S], R32[:, 0:1], None,
                                    mult)

        sweep(0)
        renorm(first=True)
        sweep(1)
        renorm(first=False)
        sweep(2)

        # ---- lse tail ----
        nc.scalar.activation(LNALL[:], SEALL[:], AF.Ln)
        SLQ = psumQ.tile([C, 1], f32, tag="slq")
        nc.tensor.matmul(SLQ[:], LNALL[:], ONESC[:], start=True, stop=True)
        nc.scalar.copy(SLQS[:], SLQ[:])      # ACT: keep the DVE queue clear
        SUMLP = psumQ.tile([BC, 1], f32, tag="sumlp")
        nc.tensor.matmul(SUMLP[:], SEL[:], SLQS[:], start=True, stop=True)
        nc.scalar.copy(SUMLSE[:], SUMLP[:])

        # ---- readout ----
        VT = persist.tile([BC, 1], f32)
        nc.vector.tensor_tensor(VT[:], SV[:, (S - 1) % NSLOT, W - 1:W],
                                SV[:, (S - 2) % NSLOT, W - 1:W], add)
        # pre-scale by 2^VPRE (ACT mishandles denormal inputs), then
        # ln(v) = 4*ln(v^(1/4)): ACT Ln clamps outside ~[2^-64, 2^64]
        VT2 = persist.tile([BC, 1], f32)
        nc.vector.tensor_scalar_mul(VT2[:], VT[:], float(2.0 ** VPRE))
        S1 = persist.tile([BC, 1], f32)
        nc.scalar.activation(S1[:], VT2[:], AF.Sqrt)
        S2 = persist.tile([BC, 1], f32)
        nc.scalar.activation(S2[:], S1[:], AF.Sqrt)
        LNQ = persist.tile([BC, 1], f32)
        nc.scalar.activation(LNQ[:], S2[:], AF.Ln)
        LOGV = persist.tile([BC, 1], f32)
        nc.vector.tensor_scalar(LOGV[:], LNQ[:], 4.0, -VPRE * LN2, mult, add)
        DEV = persist.tile([BC, 1], f32)
        nc.vector.tensor_tensor(DEV[:], LOGV[:], LOGSF[:], add)
        nc.vector.tensor_tensor(DEV[:], DEV[:], SUMLSE[:],
                                mybir.AluOpType.subtract)
        nc.sync.dma_start(out_d[:], DEV[:])
        DBG = persist.tile([BC, 8], f32)
        nc.vector.tensor_copy(DBG[:, 0:1], SUMLSE[:])
        nc.vector.tensor_copy(DBG[:, 1:2], LOGSF[:])
        nc.vector.tensor_copy(DBG[:, 2:3], LOGV[:])
        nc.vector.tensor_copy(DBG[:, 3:4], VT[:])
        nc.vector.tensor_copy(DBG[:, 4:5], M[:])
        nc.vector.tensor_copy(DBG[:, 5:6], R32[:])
        nc.vector.tensor_copy(DBG[:, 6:7], EC[:, 0:1])
        nc.vector.tensor_copy(DBG[:, 7:8], EC[:, S - 1:S])
        nc.sync.dma_start(dbg_d[:], DBG[:])

    nc.compile()
    return nc


def _host_prep(y_pred, y_true):
    import ml_dtypes
    yp = np.asarray(y_pred, dtype=np.float32)                 # [B, T, C]
    lab = np.asarray(y_true).astype(np.int32)                 # [B, L]
    xT = np.ascontiguousarray(yp.transpose(2, 0, 1)).astype(
        ml_dtypes.bfloat16)                                   # [C, B, T]
    oh = np.zeros((C, B, L), np.float32)
    oh[0, :, :] = -1.0
    oh[lab, np.arange(B)[:, None], np.arange(L)[None, :]] = 1.0
    oh = oh.astype(ml_dtypes.bfloat16)                        # [C, B, L]
    skm = np.ones((B, L), np.float32)
    skm[:, 1:] = (lab[:, 1:] != lab[:, :-1]).astype(np.float32)
    sel = (np.arange(C)[:, None] // 4
           == np.arange(BC)[None, :]).astype(np.float32)      # [128, 32]
    blanksum = yp[:, :, 0].astype(np.float64).sum(axis=1)     # [B]
    # first E chunk on host: E[b, l<LCH, t<TS] = exp(x[lab]-x[blank]+ln_kappa)
    xf = xT.astype(np.float32)                                # [C, B, T]
    g = (xf[lab[:, :LCH], np.arange(B)[:, None], :TS]
         - xf[0, :, None, :TS])                               # [B, LCH, TS]
    ebh = np.exp(g + LN_KAPPA).astype(ml_dtypes.bfloat16).reshape(B, LCH * TS)
    return xT, oh, skm, sel, blanksum, ebh


def kernel(y_pred, y_true, _trace=False):
    from concourse.bass_utils import run_bass_kernel_spmd

    xT, oh, skm, sel, blanksum, ebh = _host_prep(y_pred, y_true)
    if "nc" not in _cache:
        _cache["nc"] = _build()
    nc = _cache["nc"]

    in_maps = []
    for i in range(NCORES):
        sl = slice(i * BC, (i + 1) * BC)
        in_maps.append({"xT": np.ascontiguousarray(xT[:, sl]),
                        "oh": np.ascontiguousarray(oh[:, sl]),
                        "skm": skm[sl], "sel": sel, "ebh": ebh[sl]})
    res = run_bass_kernel_spmd(nc, in_maps, core_ids=list(range(NCORES)),
                               trace=_trace)
    _cache["last_result"] = res
    dev = np.concatenate([r["dev_out"][:, 0] for r in res.results])   # [B]
    loss = -(dev.astype(np.float64) - T * LN_KAPPA + blanksum)
    return loss.astype(np.float32)


# revision 23
# speedup vs baseline: 2.2519x; 2.2519x over previous
"""CTC loss kernel for Trainium2, 8-way data parallel over the batch.

Per core (32 examples): the CTC forward DP runs s-major — for each extended
state s (193), the time recursion is one `tensor_tensor_scan` per t-segment
on DVE, batched over examples on partitions.  T is processed as 320+192 with
one power-of-2 renorm between the segments (exponent extracted with int ALU
ops, so the renorm never touches the activation queue).  Two segments is the
minimum op count that keeps fp32 in range: the first runs from the ~2^0 start
(top stays under ~2^101), the second from a 2^44 anchor (worst-case end-state
decay ~2^-143 stays normal).

Alpha rows live in 16 rolling slots of width 514:
  col 0       = seg-A boundary (0),      cols 1..320   seg-A out (t 0..319)
  col 321     = renormed alpha[319],     cols 322..513 seg-B out (t 320..511)
A row's shifted window is contiguous inside its slot, so no per-row boundary
copies are needed; boundaries cross the renorm via one bulk capture and one
bulk scatter per 8 rows ([32,8] strided copies; 8-row batches over 16 slots
never collide with in-flight window reads).

Emissions are blank-normalized (E = exp(x[label] - x[blank] + ln_kappa)); the
log-softmax denominator cancels up to a bulk sum of per-(b,t) logsumexp done
via ACT exp + PE ones-matmuls + one ACT ln.  Gathered label logits come from
Δ-one-hot matmuls (bf16) on PE; the [l,e,t]→[e,l,t] layout flip is a batched
DRAM-roundtrip DMA per 24-row chunk (direct SBUF→SBUF permuted DMAs corrupt
data on this toolchain — do not "simplify" back to one DMA).  The first
chunk of segment A is precomputed on the host so the sweep starts without
waiting out the on-device production latency chain.
"""
import numpy as np
from contextlib import ExitStack

B, T, C, L = 256, 512, 128, 96
S = 2 * L + 1
NCORES = 8
BC = B // NCORES          # 32 examples per core
SEGS = [(0, 0, 320), (1, 320, 192)]     # (idx, t0, len)
TA, TB = 320, 192
W = 2 + TA + TB           # 514 slot width
BASES = [0, 1 + TA]       # boundary col per segment
NSLOT = 16
LCH = 24                  # emission-production chunk: label rows per chunk
NCH = L // LCH            # 4 chunks per segment
LN_KAPPA = -1.25
LN2 = float(np.log(2.0))
ROFF = 44                 # renorm target exponent: boundary max -> [2^44, 2^45)
VPRE = 96                 # readout pre-scale 2^96 so ln() sees a normal value

_cache = {}


def _build():
    import concourse.bass as bass
    import concourse.bacc as bacc
    import concourse.tile as tile
    import concourse.mybir as mybir

    f32 = mybir.dt.float32
    bf16 = mybir.dt.bfloat16
    i32 = mybir.dt.int32
    add = mybir.AluOpType.add
    mult = mybir.AluOpType.mult
    AF = mybir.ActivationFunctionType

    nc = bacc.Bacc("TRN2", target_bir_lowering=False, debug=False,
                   num_devices=NCORES)

    xT_d = nc.dram_tensor("xT", [C, BC, T], bf16, kind="ExternalInput")
    oh_d = nc.dram_tensor("oh", [C, BC, L], bf16, kind="ExternalInput")
    skm_d = nc.dram_tensor("skm", [BC, L], f32, kind="ExternalInput")
    sel_d = nc.dram_tensor("sel", [C, BC], f32, kind="ExternalInput")
    ebh_d = nc.dram_tensor("ebh", [BC, LCH * TA], bf16, kind="ExternalInput")
    out_d = nc.dram_tensor("dev_out", [BC, 1], f32, kind="ExternalOutput")
    es_d = [nc.dram_tensor(f"es{q}", [L, BC, ln], bf16, kind="Internal")
            for q, _, ln in SEGS]

    with tile.TileContext(nc, num_cores=NCORES) as tc, ExitStack() as ctx:
        persist = ctx.enter_context(tc.tile_pool(name="persist", bufs=1))
        xtpool = ctx.enter_context(tc.tile_pool(name="xt", bufs=1))
        espool = ctx.enter_context(tc.tile_pool(name="es", bufs=1))
        ebApool = ctx.enter_context(tc.tile_pool(name="ebA", bufs=3))
        ebBpool = ctx.enter_context(tc.tile_pool(name="ebB", bufs=3))
        expool = ctx.enter_context(tc.tile_pool(name="ex", bufs=1))
        upool = ctx.enter_context(tc.tile_pool(name="u", bufs=2))
        psumG = ctx.enter_context(
            tc.tile_pool(name="psumG", bufs=2, space=bass.MemorySpace.PSUM))
        psumS = ctx.enter_context(
            tc.tile_pool(name="psumS", bufs=1, space=bass.MemorySpace.PSUM))
        psumQ = ctx.enter_context(
            tc.tile_pool(name="psumQ", bufs=1, space=bass.MemorySpace.PSUM))

        # ---- persistent tiles ----
        SLOTS = persist.tile([BC, NSLOT * W], f32)
        SV = SLOTS[:].rearrange("p (n w) -> p n w", w=W)
        Z = persist.tile([BC, TA], f32)
        KT = persist.tile([BC, TA], f32)
        ONES = persist.tile([BC, 1], f32)
        SKM = persist.tile([BC, L], f32)
        EC = persist.tile([BC, S], f32)         # boundary alpha per row
        OHALL = persist.tile([C, BC * L], bf16)
        ONESC = persist.tile([C, 1], f32)
        ONESCB = persist.tile([C, 1], bf16)
        KBIAS = persist.tile([LCH, 1], f32)
        SEL = persist.tile([C, BC], f32)
        LNALL = persist.tile([C, C], f32)
        SLQS = persist.tile([C, 1], f32)
        SUMLSE = persist.tile([BC, 1], f32)
        LOGSF = persist.tile([BC, 1], f32)
        SEALL = psumS.tile([C, C], f32)         # lse: col = e*4 + chunk

        # host-precomputed first E chunk + small inputs go first on the DMA
        # queue: the sweep's first scans depend on them
        EBH = persist.tile([BC, LCH * TA], bf16)
        nc.sync.dma_start(EBH[:], ebh_d[:])
        nc.sync.dma_start(SKM[:], skm_d[:])
        nc.sync.dma_start(SEL[:], sel_d[:])
        nc.sync.dma_start(OHALL[:].rearrange("c (e l) -> c e l", l=L), oh_d[:])

        nc.vector.memset(Z[:], 0.0)
        nc.vector.memset(KT[:], float(np.exp(np.float32(LN_KAPPA))))
        nc.vector.memset(ONES[:], 1.0)
        nc.vector.memset(ONESC[:], 1.0)
        nc.vector.memset(ONESCB[:], 1.0)
        nc.vector.memset(KBIAS[:], float(LN_KAPPA))
        nc.vector.memset(SV[:, :, 0], 0.0)      # seg-A boundary cols

        XT = []
        for q, t0, ln in SEGS:
            xt = xtpool.tile([C, BC * ln], bf16, tag=f"xt{q}")
            nc.sync.dma_start(xt[:].rearrange("c (e t) -> c e t", t=ln),
                              xT_d[:, :, t0:t0 + ln])
            XT.append(xt)

        # ---- emission production: E[e, l, t] chunks via PE gather + ACT exp,
        # reshuffled [l,e,t]->[e,l,t] through DRAM ----
        EB = {(0, 0): EBH}
        for q, t0, ln in SEGS:
            pool = ebApool if q == 0 else ebBpool
            for ch in range(NCH):
                if (q, ch) == (0, 0):
                    continue
                l0 = ch * LCH
                ES = espool.tile([LCH, BC * ln], bf16, tag=f"es{q}")
                # one PSUM bank is 2 KB: at 320 cols a G tile holds one
                # example, at 192 cols a pair
                ew = 1 if ln * 8 > 2048 else 2
                for e0 in range(0, BC, ew):
                    G = psumG.tile([LCH, ew * ln], f32, tag=f"G{q}")
                    for j in range(ew):
                        e = e0 + j
                        nc.tensor.matmul(
                            G[:, j * ln:(j + 1) * ln],
                            OHALL[:, e * L + l0:e * L + l0 + LCH],
                            XT[q][:, e * ln:(e + 1) * ln],
                            start=True, stop=True)
                    nc.scalar.activation(ES[:, e0 * ln:(e0 + ew) * ln], G[:],
                                         AF.Exp, bias=KBIAS[:])
                nc.sync.dma_start(
                    es_d[q][l0:l0 + LCH],
                    ES[:].rearrange("l (e t) -> l e t", t=ln))
                eb = pool.tile([BC, LCH * ln], bf16, tag=f"eb{q}")
                nc.sync.dma_start(
                    eb[:].rearrange("e (l t) -> e l t", t=ln),
                    es_d[q][l0:l0 + LCH].rearrange("l e t -> e l t"))
                EB[(q, ch)] = eb

        # ---- lse: SEALL[t, e*4 + j] = sum_c exp(x[c, 128j + t]) ----
        # 128-t chunks; chunk 2 spans both XT tiles
        for j in range(4):
            EX = expool.tile([C, BC * C], bf16, tag="ex")
            tg0 = j * C
            for (q, t0, ln) in SEGS:
                lo = max(tg0, t0)
                hi = min(tg0 + C, t0 + ln)
                if lo >= hi:
                    continue
                nc.scalar.activation(
                    EX[:].rearrange("c (e t) -> c e t", t=C)
                        [:, :, lo - tg0:hi - tg0],
                    XT[q][:].rearrange("c (e t) -> c e t", t=ln)
                        [:, :, lo - t0:hi - t0],
                    AF.Exp)
            for e in range(BC):
                nc.tensor.matmul(
                    SEALL[:, e * 4 + j:e * 4 + j + 1],
                    EX[:, e * C:(e + 1) * C],
                    ONESCB[:], start=True, stop=True,
                    skip_group_check=True)

        # ---- the s-sweeps (all DVE) ----
        def sweep(si):
            _, t0, ln = SEGS[si]
            base = BASES[si]
            for s in range(S):
                n = s % NSLOT
                if si > 0 and s % 8 == 0:       # scatter renormed boundaries
                    hi = min(s + 8, S)
                    nc.vector.tensor_copy(SV[:, n:n + (hi - s), base],
                                          EC[:, s:hi])
                if s % 2 == 1 and ((s - 1) // 2) % LCH == 0:
                    # absorb the E-chunk DMA dependency into a TT op (the
                    # scan ISA has a single sync-wait slot)
                    ch = ((s - 1) // 2) // LCH
                    nc.vector.tensor_tensor(SKM[:, 0:1], SKM[:, 0:1],
                                            EB[(si, ch)][:, 0:1],
                                            mybir.AluOpType.bypass)
                w1 = (SV[:, (s - 1) % NSLOT, base:base + ln] if s >= 1
                      else Z[:, 0:ln])
                w2 = (SV[:, (s - 2) % NSLOT, base:base + ln] if s >= 2
                      else Z[:, 0:ln])
                init = (ONES[:, 0:1] if (si == 0 and s < 2)
                        else SV[:, n, base:base + 1])
                dst = SV[:, n, base + 1:base + 1 + ln]
                if s % 2 == 0:
                    nc.vector.tensor_tensor_scan(dst, w1, KT[:, 0:ln], init,
                                                 add, mult)
                else:
                    l = (s - 1) // 2
                    ch, lo = l // LCH, l % LCH
                    U = upool.tile([BC, TA], f32, tag="u")
                    nc.vector.scalar_tensor_tensor(U[:, 0:ln], w2,
                                                   SKM[:, l:l + 1], w1,
                                                   mult, add)
                    nc.vector.tensor_tensor_scan(
                        dst, U[:, 0:ln],
                        EB[(si, ch)][:, lo * ln:(lo + 1) * ln],
                        init, add, mult)
                if si == 0 and (s % 8 == 7 or s == S - 1):
                    lo = s - s % 8              # capture boundary col per row
                    nc.vector.tensor_copy(EC[:, lo:s + 1],
                                          SV[:, lo % NSLOT:lo % NSLOT
                                             + (s + 1 - lo), base + ln])

        sweep(0)

        # ---- power-of-2 renorm (int ALU only) ----
        M = persist.tile([BC, 1], f32)
        nc.vector.tensor_reduce(M[:], EC[:, 0:S], mybir.AxisListType.X,
                                mybir.AluOpType.max)
        EXPB = persist.tile([BC, 1], i32)
        nc.vector.tensor_scalar(EXPB[:], M[:].bitcast(i32), 23, None,
                                mybir.AluOpType.logical_shift_right)
        T2 = persist.tile([BC, 1], i32)
        nc.vector.tensor_scalar(T2[:], EXPB[:], -1, 254 + ROFF, mult, add)
        # clamp the biased exponent to 254 (r <= 2^127), else a tiny segment
        # max overflows the exponent field and r becomes inf -> 0*inf = NaN
        nc.vector.tensor_scalar(T2[:], T2[:], 254, None, mybir.AluOpType.min)
        T3 = persist.tile([BC, 1], i32)
        nc.vector.tensor_scalar(T3[:], T2[:], 23, None,
                                mybir.AluOpType.logical_shift_left)
        R32 = persist.tile([BC, 1], f32)
        nc.vector.tensor_copy(R32[:], T3[:].bitcast(f32))   # r = 2^(T2-127)
        EF = persist.tile([BC, 1], f32)
        nc.vector.tensor_copy(EF[:], T2[:])                 # int -> float
        nc.vector.tensor_scalar(LOGSF[:], EF[:], -LN2, 127.0 * LN2,
                                mult, add)                  # ln(1/r)
        nc.vector.tensor_scalar(EC[:, 0:S], EC[:, 0:S], R32[:, 0:1], None,
                                mult)

        sweep(1)

        # ---- lse tail ----
        nc.scalar.activation(LNALL[:], SEALL[:], AF.Ln)
        SLQ = psumQ.tile([C, 1], f32, tag="slq")
        nc.tensor.matmul(SLQ[:], LNALL[:], ONESC[:], start=True, stop=True)
        nc.scalar.copy(SLQS[:], SLQ[:])      # ACT: keep the DVE queue clear
        SUMLP = psumQ.tile([BC, 1], f32, tag="sumlp")
        nc.tensor.matmul(SUMLP[:], SEL[:], SLQS[:], start=True, stop=True)
        nc.scalar.copy(SUMLSE[:], SUMLP[:])

        # ---- readout ----
        VT = persist.tile([BC, 1], f32)
        nc.vector.tensor_tensor(VT[:], SV[:, (S - 1) % NSLOT, W - 1:W],
                                SV[:, (S - 2) % NSLOT, W - 1:W], add)
        # pre-scale by 2^VPRE (ACT mishandles denormal inputs), then
        # ln(v) = 4*ln(v^(1/4)): ACT Ln clamps outside ~[2^-64, 2^64]
        VT2 = persist.tile([BC, 1], f32)
        nc.vector.tensor_scalar_mul(VT2[:], VT[:], float(2.0 ** VPRE))
        S1 = persist.tile([BC, 1], f32)
        nc.scalar.activation(S1[:], VT2[:], AF.Sqrt)
        S2 = persist.tile([BC, 1], f32)
        nc.scalar.activation(S2[:], S1[:], AF.Sqrt)
        LNQ = persist.tile([BC, 1], f32)
        nc.scalar.activation(LNQ[:], S2[:], AF.Ln)
        LOGV = persist.tile([BC, 1], f32)
        nc.vector.tensor_scalar(LOGV[:], LNQ[:], 4.0, -VPRE * LN2, mult, add)
        DEV = persist.tile([BC, 1], f32)
        nc.vector.tensor_tensor(DEV[:], LOGV[:], LOGSF[:], add)
        nc.vector.tensor_tensor(DEV[:], DEV[:], SUMLSE[:],
                                mybir.AluOpType.subtract)
        nc.sync.dma_start(out_d[:], DEV[:])

    nc.compile()
    return nc


def _host_prep(y_pred, y_true):
    import ml_dtypes
    yp = np.asarray(y_pred, dtype=np.float32)                 # [B, T, C]
    lab = np.asarray(y_true).astype(np.int32)                 # [B, L]
    xT = np.ascontiguousarray(yp.transpose(2, 0, 1)).astype(
        ml_dtypes.bfloat16)                                   # [C, B, T]
    oh = np.zeros((C, B, L), np.float32)
    oh[0, :, :] = -1.0
    oh[lab, np.arange(B)[:, None], np.arange(L)[None, :]] = 1.0
    oh = oh.astype(ml_dtypes.bfloat16)                        # [C, B, L]
    skm = np.ones((B, L), np.float32)
    skm[:, 1:] = (lab[:, 1:] != lab[:, :-1]).astype(np.float32)
    sel = (np.arange(C)[:, None] // 4
           == np.arange(BC)[None, :]).astype(np.float32)      # [128, 32]
    blanksum = yp[:, :, 0].astype(np.float64).sum(axis=1)     # [B]
    # first E chunk on host: E[b, l<LCH, t<TA] = exp(x[lab]-x[blank]+ln_kappa)
    xf = xT.astype(np.float32)                                # [C, B, T]
    g = (xf[lab[:, :LCH], np.arange(B)[:, None], :TA]
         - xf[0, :, None, :TA])                               # [B, LCH, TA]
    ebh = np.exp(g + LN_KAPPA).astype(ml_dtypes.bfloat16).reshape(B, LCH * TA)
    return xT, oh, skm, sel, blanksum, ebh


def kernel(y_pred, y_true, _trace=False):
    from concourse.bass_utils import run_bass_kernel_spmd

    xT, oh, skm, sel, blanksum, ebh = _host_prep(y_pred, y_true)
    if "nc" not in _cache:
        _cache["nc"] = _build()
    nc = _cache["nc"]

    in_maps = []
    for i in range(NCORES):
        sl = slice(i * BC, (i + 1) * BC)
        in_maps.append({"xT": np.ascontiguousarray(xT[:, sl]),
                        "oh": np.ascontiguousarray(oh[:, sl]),
                        "skm": skm[sl], "sel": sel, "ebh": ebh[sl]})
    res = run_bass_kernel_spmd(nc, in_maps, core_ids=list(range(NCORES)),
                               trace=_trace)
    _cache["last_result"] = res
    dev = np.concatenate([r["dev_out"][:, 0] for r in res.results])   # [B]
    loss = -(dev.astype(np.float64) - T * LN_KAPPA + blanksum)
    return loss.astype(np.float32)
